# revision 1
# baseline (speedup 1.0000x reference)
"""Trainium2 Bass kernel for nn_BaseNet_75256416960712 (gnn_message_passing).

Data-parallel over batch B=64 across 8 NeuronCores (8 batches per core).

Math (algebraically identical to the reference, validated to ~1e-5 rel):
  - BN1's mean/shift cancels in BN2 (BN2 subtracts its own mean), so only the
    BN1 scale a = g_inp * rsqrt(var_x + eps) survives. var_x comes from global
    second moments of s, accumulated on the PE in bf16 hi/lo split form
    (C = C_hh + C_hl + C_lh, exact to ~1e-5), then AllGather + local reduce.
  - The per-position head dot products commute with the neighbor gather:
    y_h = s @ v_h with v_h = W_feat @ (a*w_h); the gather then moves scalars,
    implemented as one-hot matmuls on the PE (onehot exact in bf16, y split
    hi/lo bf16 accumulated via the free dim). Two k-slots are stacked per
    matmul (M=128) and both heads + both splits ride the free dim (N=96).
  - P lives in a parity layout [128 = (k%2)*64 + n, b, t, k//2] so every
    DVE/ACT op downstream uses all 128 lanes; eps/dis are host-permuted to
    match.
  - BN2 stats: per-partition bn_stats/bn_aggr, AllGather, combined across
    ranks and parity halves via the law of total variance (equal counts).
    tanh/exp fused with the BN2 affine on ACT.
"""

import sys

if "/opt/trn_rl_repo" not in sys.path:
    sys.path.insert(0, "/opt/trn_rl_repo")

import numpy as np

B, T, N, D, H, MN = 64, 24, 64, 32, 64, 15
NC = 8          # cores
NB = B // NC    # batches per core
POS = NB * T * N  # 12288 positions per core
BN_EPS = 1e-5
SIGMA_MIN, SIGMA_MAX = -20.0, 2.0
MAGIC = 0x5F3759DF
M_GLOBAL = float(B * T * N)   # BN1 stat count

_CACHE = {}


def _emit_rsqrt(nc, mybir, sb, dst, src, p, w):
    """dst = rsqrt(src + BN_EPS) on [p, w] f32 tiles via bit trick + 2 Newton."""
    u = sb.tile([p, w], mybir.dt.float32, tag=f"rsq_u{w}", name=f"rsq_u{p}_{w}")
    nc.vector.tensor_scalar_add(u[:], src, BN_EPS)
    magic = sb.tile([p, w], mybir.dt.int32, tag=f"rsq_m{w}", name=f"rsq_m{p}_{w}")
    nc.vector.memset(magic[:], MAGIC)
    sh = sb.tile([p, w], mybir.dt.int32, tag=f"rsq_s{w}", name=f"rsq_s{p}_{w}")
    nc.vector.tensor_scalar(sh[:], u[:].bitcast(mybir.dt.int32), 1, None,
                            op0=mybir.AluOpType.logical_shift_right)
    y0 = sb.tile([p, w], mybir.dt.float32, tag=f"rsq_y{w}", name=f"rsq_y{p}_{w}")
    nc.vector.tensor_tensor(y0[:].bitcast(mybir.dt.int32), magic[:], sh[:],
                            op=mybir.AluOpType.subtract)
    t1 = sb.tile([p, w], mybir.dt.float32, tag=f"rsq_t{w}", name=f"rsq_t{p}_{w}")
    for it in range(2):
        out = dst if it == 1 else y0[:]
        nc.vector.tensor_tensor(t1[:], y0[:], y0[:], op=mybir.AluOpType.mult)
        nc.vector.tensor_tensor(t1[:], t1[:], u[:], op=mybir.AluOpType.mult)
        nc.vector.tensor_scalar(t1[:], t1[:], -0.5, 1.5,
                                op0=mybir.AluOpType.mult, op1=mybir.AluOpType.add)
        nc.vector.tensor_tensor(out, y0[:], t1[:], op=mybir.AluOpType.mult)


def _build():
    import concourse.bacc as bacc
    import concourse.tile as tile
    import concourse.mybir as mybir

    nc = bacc.Bacc("TRN2", target_bir_lowering=False, debug=False, num_devices=NC)
    f32 = mybir.dt.float32
    bf16 = mybir.dt.bfloat16
    Alu = mybir.AluOpType
    Act = mybir.ActivationFunctionType
    X = mybir.AxisListType.X

    s_in = nc.dram_tensor("s", [POS, D], f32, kind="ExternalInput")
    kbc_in = nc.dram_tensor("kbc", [128, NB * N * 16 // 2], bf16, kind="ExternalInput")
    eps_in = nc.dram_tensor("eps", [128, NB, 192], f32, kind="ExternalInput")
    w_in = nc.dram_tensor("W", [D, H], f32, kind="ExternalInput")
    pv_in = nc.dram_tensor("pvec", [7, H], f32, kind="ExternalInput")
    dis_out = nc.dram_tensor("dis", [128, NB, 192], f32, kind="ExternalOutput")

    with tile.TileContext(nc) as tc:
        with tc.tile_pool(name="sb", bufs=1) as sb, \
             tc.tile_pool(name="ps", bufs=2, space="PSUM") as ps, \
             tc.tile_pool(name="psg", bufs=3, space="PSUM") as psg, \
             tc.tile_pool(name="dram", bufs=1, space="DRAM") as dram:

            # ---- ACT table warmup (exp/tanh) as early as possible
            warm = sb.tile([1, 1], f32)
            nc.vector.memset(warm[:], 0.0)
            nc.scalar.activation(warm[:], warm[:], Act.Exp)
            nc.scalar.activation(warm[:], warm[:], Act.Tanh)

            # ---- s load (contiguous, chunk-partitioned) -- first on the sync ring
            s_sb = sb.tile([128, 96, D], f32)
            s_src = s_in[:].rearrange("(p k) d -> p k d", p=128)
            for j in range(4):
                nc.sync.dma_start(s_sb[:, 24 * j:24 * (j + 1), :],
                                  s_src[:, 24 * j:24 * (j + 1), :])
            # ---- params: one packed [64, 7] DMA + W
            W_sb = sb.tile([D, H], f32)
            nc.sync.dma_start(W_sb[:], w_in[:])
            pvec = sb.tile([H, 7], f32)
            nc.sync.dma_start(pvec[:], pv_in[:].rearrange("a b -> b a"))
            g_inp_c = pvec[:, 0:1]
            w2 = pvec[:, 1:3]       # [w_mu | w_lv]
            g2 = pvec[:, 3:5]       # [g_mu | g_lv]
            be2 = pvec[:, 5:7]      # [be_mu | be_lv]

            # ---- identities
            ones32 = sb.tile([D, D], f32)
            nc.vector.memset(ones32[:], 1.0)
            id32 = sb.tile([D, D], f32)
            nc.gpsimd.affine_select(id32[:], ones32[:], pattern=[[1, D]],
                                    compare_op=Alu.is_equal, fill=0.0,
                                    base=0, channel_multiplier=-1)
            # twohot [128, 64]: 1 where p % 64 == n  (parity-pair combiner)
            onesA = sb.tile([128, N], f32)
            nc.vector.memset(onesA[:], 1.0)
            twohot = sb.tile([128, N], f32)
            for g in range(2):
                nc.gpsimd.affine_select(twohot[64 * g:64 * g + 64, :],
                                        onesA[64 * g:64 * g + 64, :],
                                        pattern=[[1, N]],
                                        compare_op=Alu.is_equal, fill=0.0,
                                        base=0, channel_multiplier=-1)
            # rep [64, 128]: 1 where c % 64 == p  (64 -> 128 replicator)
            onesB = sb.tile([N, 128], f32)
            nc.vector.memset(onesB[:], 1.0)
            rep = sb.tile([N, 128], f32)
            for g in range(2):
                nc.gpsimd.affine_select(rep[:, 64 * g:64 * g + 64],
                                        onesB[:, 64 * g:64 * g + 64],
                                        pattern=[[1, N]],
                                        compare_op=Alu.is_equal, fill=0.0,
                                        base=0, channel_multiplier=-1)

            # ---- second copy of s, n'-partitioned (SWDGE ring, overlaps sync ring)
            s2 = sb.tile([128, 96, D], f32)
            ss = s_in[:].rearrange("(b t n) d -> n (b t) d", b=NB, t=T, n=N)
            nc.gpsimd.dma_start(s2[0:64, :, :], ss[:, 0:96, :])
            nc.gpsimd.dma_start(s2[64:128, :, :], ss[:, 96:192, :])

            # ---- eps load (host pre-permuted to parity layout; SWDGE ring)
            eps_sb = sb.tile([128, NB, 192], f32)
            nc.gpsimd.dma_start(eps_sb[:], eps_in[:])

            # ---- k_nei (host pre-broadcast to [128, 4096]) -> one-hot
            # on the scalar HWDGE ring so it doesn't queue behind s
            kb_sb = sb.tile([128, 4096], bf16)
            nc.scalar.dma_start(kb_sb[:], kbc_in[:])
            io = sb.tile([128, 1], mybir.dt.int32)
            nc.gpsimd.iota(io[0:64, :], pattern=[[0, 1]], base=0, channel_multiplier=1)
            nc.gpsimd.iota(io[64:128, :], pattern=[[0, 1]], base=0, channel_multiplier=1)
            iof = sb.tile([128, 1], bf16)
            nc.vector.tensor_copy(iof[:], io[:])
            oh_sb = sb.tile([128, 4096], bf16)
            oh_inst = nc.vector.tensor_tensor(oh_sb[:], kb_sb[:],
                                              iof[:].broadcast_to([128, 4096]),
                                              op=Alu.is_equal)

            # ---- W^T (needs only W; schedule before the collective wait)
            wt_ps = ps.tile([H, D], f32, tag="tiny", name="wt_ps")
            nc.tensor.transpose(wt_ps[:], W_sb[:], id32[:])
            wt_sb = sb.tile([H, D], f32)
            nc.vector.tensor_copy(wt_sb[:], wt_ps[:])

            # ---- bf16 hi/lo prep of s + moments matmuls, pipelined in 4 groups
            # s_hl[p, chunk, 0, :] = [s_hi | 1], s_hl[p, chunk, 1, :] = [s_lo | 0]
            # mom accumulates [s_hi|1]^T [s_hi|1 | s_lo|0]  ->  [33, 2, 33]
            s_hl = sb.tile([128, 96, 2, D + 1], bf16)
            nc.vector.memset(s_hl[:, :, 0, D:D + 1], 1.0)
            nc.vector.memset(s_hl[:, :, 1, D:D + 1], 0.0)
            s_rem = sb.tile([128, 96, D], f32)
            mom_ps = ps.tile([D + 1, 2, D + 1], f32, tag="mom")
            for j in range(4):
                sl = slice(24 * j, 24 * (j + 1))
                cast_inst = nc.vector.tensor_copy(s_hl[:, sl, 0, 0:D], s_sb[:, sl, :])
                if j == 0:
                    tile.add_dep_helper(cast_inst.ins, oh_inst.ins, sync=False)
                nc.vector.tensor_tensor(s_rem[:, sl, :], s_sb[:, sl, :],
                                        s_hl[:, sl, 0, 0:D], op=Alu.subtract)
                nc.vector.tensor_copy(s_hl[:, sl, 1, 0:D], s_rem[:, sl, :])
                for c in range(24 * j, 24 * (j + 1)):
                    nc.tensor.matmul(mom_ps[:].rearrange("p a b -> p (a b)"),
                                     s_hl[:, c, 0, :],
                                     s_hl[:, c, :, :].rearrange("p a b -> p (a b)"),
                                     start=(c == 0), stop=(c == 95),
                                     skip_group_check=True)
            mom_sb = sb.tile([D + 1, 2, D + 1], f32)
            nc.vector.tensor_copy(mom_sb[:], mom_ps[:])

            # ---- AllGather 1 (moments) + local reduce
            agin1 = dram.tile([D + 1, 2 * (D + 1)], f32)
            agout1 = dram.tile([NC, D + 1, 2 * (D + 1)], f32)
            nc.sync.dma_start(agin1[:], mom_sb[:].rearrange("p a b -> p (a b)"))
            nc.gpsimd.collective_compute(
                "AllGather", Alu.bypass, ins=[agin1.opt()], outs=[agout1.opt()],
                replica_groups=[list(range(NC))])
            tg1 = sb.tile([D + 1, NC, 2 * (D + 1)], f32)
            nc.sync.dma_start(tg1[:], agout1[:].rearrange("r p c -> p r c"))
            momg = sb.tile([D + 1, 2, D + 1], f32)
            nc.vector.tensor_reduce(momg[:].rearrange("p a b -> p (a b)"),
                                    tg1[:].rearrange("p r c -> p c r"),
                                    axis=X, op=Alu.add)

            # ---- BN1 scale + head vectors v
            # msum[d] = sum_hi (col D of mom_hh) + sum_lo (row D of mom_hl -> col)
            mlo_ps = ps.tile([D, 1], f32, tag="tiny", name="mlo_ps")
            nc.tensor.matmul(mlo_ps[:], momg[D:D + 1, 1, 0:D], onesA[D:D + 1, 0:1],
                             start=True, stop=True)
            msum = sb.tile([D, 1], f32)
            nc.vector.tensor_tensor(msum[:], momg[0:D, 0, D:D + 1], mlo_ps[:],
                                    op=Alu.add)
            m0_ps = ps.tile([H, 1], f32, tag="tiny")
            nc.tensor.matmul(m0_ps[:], W_sb[:], msum[:], start=True, stop=True)
            mean0r = sb.tile([H, 1], f32)
            nc.vector.tensor_copy(mean0r[:], m0_ps[:])
            # CW = (C_hh + 2*C_hl) @ W; var uses diag(W^T . W) of that sum
            cw_ps = ps.tile([D, H], f32, tag="tiny")
            nc.tensor.matmul(cw_ps[:], momg[0:D, 0, 0:D], W_sb[:], start=True, stop=False)
            nc.tensor.matmul(cw_ps[:], momg[0:D, 1, 0:D], W_sb[:], start=False, stop=False)
            nc.tensor.matmul(cw_ps[:], momg[0:D, 1, 0:D], W_sb[:], start=False, stop=True)
            tw = sb.tile([D, H], f32)
            nc.vector.tensor_tensor(tw[:], W_sb[:], cw_ps[:], op=Alu.mult)
            ex2_ps = ps.tile([H, 1], f32, tag="tiny")
            nc.tensor.matmul(ex2_ps[:], tw[:], ones32[:, 0:1], start=True, stop=True)

            varx = sb.tile([H, 1], f32)
            m0 = sb.tile([H, 1], f32)
            nc.vector.tensor_scalar_mul(m0[:], mean0r[:], 1.0 / M_GLOBAL)
            nc.vector.tensor_tensor(m0[:], m0[:], m0[:], op=Alu.mult)   # mean0^2
            nc.vector.tensor_scalar_mul(varx[:], ex2_ps[:], 1.0 / M_GLOBAL)
            nc.vector.tensor_tensor(varx[:], varx[:], m0[:], op=Alu.subtract)
            r1 = sb.tile([H, 1], f32)
            _emit_rsqrt(nc, mybir, sb, r1[:], varx[:], H, 1)
            aw2 = sb.tile([H, 2], f32)
            nc.vector.tensor_tensor(aw2[:, 0:1], g_inp_c[:], r1[:], op=Alu.mult)  # a
            nc.vector.tensor_tensor(aw2[:, 1:2], aw2[:, 0:1], w2[:, 1:2], op=Alu.mult)
            nc.vector.tensor_tensor(aw2[:, 0:1], aw2[:, 0:1], w2[:, 0:1], op=Alu.mult)

            v2_ps = ps.tile([D, 2], f32, tag="tiny")
            nc.tensor.matmul(v2_ps[:], wt_sb[:], aw2[:], start=True, stop=True)
            v2_sb = sb.tile([D, 2], f32)
            nc.vector.tensor_copy(v2_sb[:], v2_ps[:])
            vpat = []
            for h in range(2):
                vr_ps = ps.tile([1, D], f32, tag="tiny", name=f"vr_ps{h}")
                nc.tensor.matmul(vr_ps[:], v2_sb[:, h:h + 1], id32[:], start=True, stop=True)
                vr = sb.tile([1, D], f32, tag=f"vrow{h}", name=f"vrow{h}")
                nc.vector.tensor_copy(vr[:], vr_ps[:])
                vp = sb.tile([128, D], f32, tag=f"vpat{h}", name=f"vpat{h}")
                nc.gpsimd.partition_broadcast(vp[:], vr[:])
                vpat.append(vp)

            # ---- y = s @ v for both heads (f32), then hi/lo bf16 split
            y2 = sb.tile([128, 2, 96], f32)
            sv = sb.tile([128, 96, D], f32)
            svb = sb.tile([128, 96, D], f32)
            nc.gpsimd.tensor_tensor(
                svb[:], s2[:], vpat[1][:].unsqueeze(1).broadcast_to([128, 96, D]),
                op=Alu.mult)
            nc.vector.tensor_tensor(
                sv[:], s2[:], vpat[0][:].unsqueeze(1).broadcast_to([128, 96, D]),
                op=Alu.mult)
            nc.vector.tensor_reduce(y2[:, 0, :], sv[:], axis=X, op=Alu.add)
            nc.vector.tensor_reduce(y2[:, 1, :], svb[:], axis=X, op=Alu.add)
            # y2bf[p, bl, split, head, t] so the rhs slice per (bl, split) is
            # one contiguous 48-col run
            y2bf = sb.tile([128, 4, 2, 2, T], bf16)
            yrem = sb.tile([128, 2, 96], f32)
            hi_v = y2bf[:, :, 0, :, :].rearrange("p bl h t -> p h bl t")
            lo_v = y2bf[:, :, 1, :, :].rearrange("p bl h t -> p h bl t")
            y2_v = y2[:].rearrange("p h (bl t) -> p h bl t", bl=4)
            yrem_v = yrem[:].rearrange("p h (bl t) -> p h bl t", bl=4)
            nc.vector.tensor_copy(hi_v, y2_v)
            nc.vector.tensor_tensor(yrem_v, y2_v, hi_v, op=Alu.subtract)
            nc.vector.tensor_copy(lo_v, yrem_v)

            # ---- gather: one matmul per (b, k-pair); out rows = kslot*64 + n
            # P2[h][p = (k%2)*64 + n, b, t, k//2]
            P2 = [sb.tile([128, NB, T, 8], f32, tag=f"P{h}", name=f"P{h}")
                  for h in range(2)]
            for b in range(NB):
                half, bl = b // 4, b % 4
                prow = slice(64 * half, 64 * half + 64)
                pb = psg.tile([128, 8, 2 * T], f32, tag="gps", name=f"gps_{b}")
                for j2 in range(8):
                    lhsT = oh_sb[prow, 1024 * bl + 128 * j2:1024 * bl + 128 * j2 + 128]
                    nc.tensor.matmul(pb[:, j2, :], lhsT, y2bf[prow, bl, 0, :, :],
                                     start=True, stop=False, skip_group_check=True)
                    nc.tensor.matmul(pb[:, j2, :], lhsT, y2bf[prow, bl, 1, :, :],
                                     start=False, stop=True, skip_group_check=True)
                # copy out per head: psum [p, j2, head*T + t] -> P2[p, b, t, j2]
                for h in range(2):
                    nc.vector.tensor_copy(
                        P2[h][:, b, :, :].rearrange("p t j -> p j t"),
                        pb[:, :, T * h:T * h + T])

            # ---- BN2 per-core stats (per partition = (parity, n))
            stats2 = sb.tile([128, 4], f32)
            for h in range(2):
                bns = sb.tile([128, 3, 6], f32, tag=f"bns{h}", name=f"bns{h}")
                Pv = P2[h][:].rearrange("p b t j -> p (b t j)").rearrange(
                    "p (s c) -> p s c", c=512)
                for sch in range(3):
                    nc.vector.bn_stats(bns[:, sch, :], Pv[:, sch, :])
                nc.vector.bn_aggr(stats2[:, 2 * h:2 * h + 2],
                                  bns[:].rearrange("p s c -> p (s c)"))

            # ---- AllGather 2 (BN2 stats) + combine over ranks and parity
            agin2 = dram.tile([128, 4], f32)
            agout2 = dram.tile([NC, 128, 4], f32)
            nc.sync.dma_start(agin2[:], stats2[:])
            nc.gpsimd.collective_compute(
                "AllGather", Alu.bypass, ins=[agin2.opt()], outs=[agout2.opt()],
                replica_groups=[list(range(NC))])
            tg2 = sb.tile([128, NC, 4], f32)
            nc.sync.dma_start(tg2[:], agout2[:].rearrange("r p c -> p r c"))
            tv = tg2[:].rearrange("p r c -> p c r")
            statr = sb.tile([128, 6], f32)   # msum, vsum, msq per head
            sqm = sb.tile([128, NC], f32)
            for h in range(2):
                nc.vector.tensor_reduce(statr[:, 3 * h:3 * h + 1],
                                        tv[:, 2 * h:2 * h + 1, :], axis=X, op=Alu.add)
                nc.vector.tensor_reduce(statr[:, 3 * h + 1:3 * h + 2],
                                        tv[:, 2 * h + 1:2 * h + 2, :], axis=X, op=Alu.add)
                nc.vector.tensor_tensor(sqm[:], tg2[:, :, 2 * h], tg2[:, :, 2 * h],
                                        op=Alu.mult)
                nc.vector.tensor_reduce(statr[:, 3 * h + 2:3 * h + 3], sqm[:],
                                        axis=X, op=Alu.add)
            comb_ps = ps.tile([N, 6], f32, tag="tiny")
            nc.tensor.matmul(comb_ps[:], twohot[:], statr[:], start=True, stop=True)
            comb = sb.tile([N, 6], f32)
            nc.vector.tensor_scalar_mul(comb[:], comb_ps[:], 1.0 / 16.0)
            mg2 = sb.tile([N, 2], f32)
            vg2 = sb.tile([N, 2], f32)
            for h in range(2):
                nc.vector.tensor_copy(mg2[:, h:h + 1], comb[:, 3 * h:3 * h + 1])
                nc.vector.tensor_tensor(vg2[:, h:h + 1], comb[:, 3 * h + 1:3 * h + 2],
                                        comb[:, 3 * h + 2:3 * h + 3], op=Alu.add)
            msq2 = sb.tile([N, 2], f32)
            nc.vector.tensor_tensor(msq2[:], mg2[:], mg2[:], op=Alu.mult)
            nc.vector.tensor_tensor(vg2[:], vg2[:], msq2[:], op=Alu.subtract)

            r2 = sb.tile([N, 2], f32)
            _emit_rsqrt(nc, mybir, sb, r2[:], vg2[:], N, 2)
            scale2 = sb.tile([N, 2], f32)
            nc.vector.tensor_tensor(scale2[:], g2[:], r2[:], op=Alu.mult)
            shift2 = sb.tile([N, 2], f32)
            nc.vector.tensor_tensor(shift2[:], mg2[:], scale2[:], op=Alu.mult)
            nc.vector.tensor_tensor(shift2[:], be2[:], shift2[:], op=Alu.subtract)
            # clip bounds in P-space: lo = (SIGMA_MIN - shift)/scale etc.
            inv_s = sb.tile([N, 1], f32)
            nc.vector.reciprocal(inv_s[:], scale2[:, 1:2])
            lohi = sb.tile([N, 2], f32)
            nc.vector.tensor_scalar(lohi[:, 0:1], shift2[:, 1:2], -1.0, SIGMA_MIN,
                                    op0=Alu.mult, op1=Alu.add)
            nc.vector.tensor_scalar(lohi[:, 1:2], shift2[:, 1:2], -1.0, SIGMA_MAX,
                                    op0=Alu.mult, op1=Alu.add)
            nc.vector.tensor_tensor(lohi[:], lohi[:],
                                    inv_s[:].broadcast_to([N, 2]), op=Alu.mult)
            # replicate all 6 per-channel consts to 128 partitions via rep matmul
            cons = sb.tile([N, 6], f32)
            for i, (t_, c_) in enumerate([(scale2, 0), (shift2, 0), (scale2, 1),
                                          (shift2, 1), (lohi, 0), (lohi, 1)]):
                nc.vector.tensor_copy(cons[:, i:i + 1], t_[:, c_:c_ + 1])
            rep_ps = ps.tile([128, 6], f32, tag="tiny")
            nc.tensor.matmul(rep_ps[:], rep[:], cons[:], start=True, stop=True)
            repc = sb.tile([128, 6], f32)
            nc.vector.tensor_copy(repc[:], rep_ps[:])

            # ---- tail: mu = tanh(affine(P0)); var = exp(affine(clip(P1)))
            mu_sb = sb.tile([128, NB, 192], f32)
            nc.scalar.activation(mu_sb[:].rearrange("p b c -> p (b c)"),
                                 P2[0][:].rearrange("p b t j -> p (b t j)"),
                                 Act.Tanh, bias=repc[:, 1:2], scale=repc[:, 0:1])
            tcl = sb.tile([128, NB, 192], f32)
            nc.vector.tensor_scalar(tcl[:].rearrange("p b c -> p (b c)"),
                                    P2[1][:].rearrange("p b t j -> p (b t j)"),
                                    repc[:, 4:5], repc[:, 5:6],
                                    op0=Alu.max, op1=Alu.min)
            var_sb = sb.tile([128, NB, 192], f32)
            nc.scalar.activation(var_sb[:].rearrange("p b c -> p (b c)"),
                                 tcl[:].rearrange("p b c -> p (b c)"),
                                 Act.Exp, bias=repc[:, 3:4], scale=repc[:, 2:3])

            dis_sb = sb.tile([128, NB, 192], f32)
            nc.vector.tensor_tensor(dis_sb[:], eps_sb[:], var_sb[:], op=Alu.mult)
            nc.vector.tensor_tensor(dis_sb[:], dis_sb[:], mu_sb[:], op=Alu.add)
            nc.sync.dma_start(dis_out[:], dis_sb[:])

    nc.compile()
    return nc


def kernel(**inputs):
    import ml_dtypes
    from concourse.bass_utils import run_bass_kernel_spmd

    if "nc" not in _CACHE:
        _CACHE["nc"] = _build()
    nc = _CACHE["nc"]

    s = np.ascontiguousarray(np.asarray(inputs["s"], dtype=np.float32))
    eps = np.ascontiguousarray(np.asarray(inputs["eps"], dtype=np.float32))
    k_nei = np.asarray(inputs["k_nei"]).astype(np.float32)
    W = np.ascontiguousarray(np.asarray(inputs["W_feat"], dtype=np.float32))
    pvec = np.ascontiguousarray(np.stack(
        [np.asarray(inputs[n], dtype=np.float32)
         for n in ["g_inp", "w_mu", "w_lv", "g_mu", "g_lv", "be_mu", "be_lv"]]))

    # augment k_nei with the self index as k=0: kfull[b, n, 16]
    self_idx = np.broadcast_to(np.arange(N, dtype=np.float32)[None, :, None],
                               (B, N, 1))
    kfull = np.concatenate([self_idx, k_nei], axis=2)
    # reorder to (b, j2, kslot, n) so each k-pair one-hot block is contiguous
    kfull = np.ascontiguousarray(
        kfull.reshape(B, N, 8, 2).transpose(0, 2, 3, 1)).astype(ml_dtypes.bfloat16)

    in_maps = []
    for c in range(NC):
        bsl = slice(NB * c, NB * (c + 1))
        # eps -> parity layout [128 = (k%2)*64 + n, b, t*8 + k//2]
        e = eps[bsl].reshape(NB, N, T, 8, 2).transpose(4, 1, 0, 2, 3)
        kb = np.broadcast_to(kfull[bsl].reshape(2, 1, 4096), (2, 64, 4096))
        in_maps.append({
            "s": s[bsl].reshape(POS, D),
            "kbc": np.ascontiguousarray(kb.reshape(128, 4096)),
            "eps": np.ascontiguousarray(e.reshape(128, NB, 192)),
            "W": W,
            "pvec": pvec,
        })
    res = run_bass_kernel_spmd(nc, in_maps, core_ids=list(range(NC)))
    out = np.empty((B, N, T, 16), np.float32)
    for c in range(NC):
        d = res.results[c]["dis"].reshape(2, N, NB, T, 8)
        out[NB * c: NB * (c + 1)] = d.transpose(2, 1, 3, 4, 0).reshape(NB, N, T, 16)
    return np.ascontiguousarray(out)

